# revision 11
# baseline (speedup 1.0000x reference)
# Trainium2 Bass kernel for nn_MeshUnpool (gnn_message_passing).
#
# Reference semantics (per mesh b):
#   idx = cumsum(dst_mask)-1 at true slots; padded[v,:] = mask[v] ? features[:,idx[v]] : 0
#   out = (unroll_mat[b].T @ padded).T / occ  ==  (features[b] @ unroll_mat[b][mask_rows]) / occ
# i.e. the gather+scatter collapses to selecting the E=3072 masked rows of
# unroll_mat, leaving a dense [NF,E] @ [E,U] matmul per mesh, divided
# column-wise by occurrences.  Pure data parallel: one mesh per core.
#
# On-device compute per core:
#   out[128, ncol] = sum_k A[k].T @ W[k]
# A = features^T in fp16 (stationary), W = the masked-row gather of
# unroll_mat cast to fp8 (entries 0/1, lossless; moving operand at 1
# byte/elem).  The PE runs 1 cycle per output column per 128-row chunk for
# any <=16-bit dtype pair, so a single fp16 pass costs half of what the old
# bf16 hi/lo split paid.
#
# Memory-bound regime, ~12.4 MB/core HBM traffic:
#   - all-zero ROWS of W dropped on host (~6%)
#   - all-zero COLUMNS of W dropped on host (~11%); host scatters computed
#     columns back into the full [128, 4096] output
#   - occurrences never shipped: division happens on host after the gather
#   - out ships fp16 (0.92 MB)
#   - each chunk's 256-byte-per-partition fp16 stationary is PACKED IN FRONT
#     of its W rows and read through a bitcast AP, so one DMA per chunk
#     delivers both and the tile scheduler cannot reorder A behind W
#   - the W stream alternates between the two HWDGE queues (SP + Act)
#   - ~2.5us of warmup matmuls on a zeroed tile ramp the PE out of its cold
#     p-state while the first W chunk streams in
#   - redundant LDWEIGHTS are stripped post-compile (_dedup_ldweights)

import numpy as np
import ml_dtypes

B, NF, E, U = 8, 128, 3072, 4096
NCORES = 8
AB = 256  # stationary bytes per partition packed ahead of each W chunk row

_compiled = {}


def _build_bass(kc, ncol):
    """Per-core program: kc 128-row chunks contracted over ncol kept output
    columns, one fp16(stationary) x fp8(moving) matmul pass per chunk."""
    import concourse.bass as bass
    import concourse.bacc as bacc
    import concourse.mybir as mybir
    import concourse.tile as tile

    nc = bacc.Bacc("TRN2", target_bir_lowering=False, debug=False)
    fp8 = mybir.dt.float8e4
    f16 = mybir.dt.float16
    f32 = mybir.dt.float32

    rowb = AB + ncol  # bytes per partition per chunk: [fp16 A | fp8 W]
    w = nc.dram_tensor("w", [128, kc, rowb], fp8, kind="ExternalInput").ap()
    out = nc.dram_tensor("out", [128, ncol], f16, kind="ExternalOutput").ap()

    # PSUM column tiles of up to 1024 (2 banks); matmul slices of up to 512
    # (1 bank) never cross a bank boundary.
    ptiles = []
    off = 0
    while off < ncol:
        wd = min(1024, ncol - off)
        ptiles.append((off, wd))
        off += wd
    slices = []
    off = 0
    while off < ncol:
        wd = min(512, ncol - off)
        slices.append((off, wd))
        off += wd

    def locate(coff):
        for i, (o, wd) in enumerate(ptiles):
            if o <= coff < o + wd:
                return i, coff - o
        raise AssertionError(coff)

    with tile.TileContext(nc) as tc:
        with (
            tc.tile_pool(name="zpool", bufs=1) as zpool,
            tc.tile_pool(name="wpool", bufs=6) as wpool,
            tc.tile_pool(name="psum", bufs=1, space=bass.MemorySpace.PSUM) as ppool,
            tc.tile_pool(name="opool", bufs=4) as opool,
        ):
            z_t = zpool.tile([128, 512], fp8, tag="z")
            psums = [
                ppool.tile([128, wd], f32, tag=f"ps{i}", name=f"ps{i}")
                for i, (o, wd) in enumerate(ptiles)
            ]

            # PE p-state warmup: ~5us of throwaway matmuls on a zeroed tile
            # run while the W prologue streams (the first chunk takes that
            # long to arrive anyway: trigger chain + DGE ramp + sem prop), so
            # the real stream starts at the full 2.4 GHz clock and never
            # exposes the cold 0.65 GHz p-state.
            nc.vector.memset(z_t[:], 0)
            for _ in range(8):
                nc.tensor.matmul(
                    psums[0][:, 0:512], z_t[:, 0:128], z_t[:], start=True, stop=True
                )

            def mm(k, w_t, coff, cw, start, stop):
                ti, lo = locate(coff)
                nc.tensor.matmul(
                    psums[ti][:, lo : lo + cw],
                    w_t[:, 0:AB].bitcast(f16),
                    w_t[:, AB + coff : AB + coff + cw],
                    start=start,
                    stop=stop,
                )

            for k in range(kc):
                w_t = wpool.tile([128, rowb], fp8, tag="w")
                qe = nc.sync if k % 2 == 0 else nc.scalar
                if k == 0:
                    # prologue: first chunk split so the first matmul waits on
                    # only ~96KB (stationary + first 512 W columns)
                    c1, c2 = AB + 512, AB + 1536
                    nc.sync.dma_start(w_t[:, 0:c1], w[:, 0, 0:c1])
                    nc.sync.dma_start(w_t[:, c1:c2], w[:, 0, c1:c2])
                    nc.sync.dma_start(w_t[:, c2:rowb], w[:, 0, c2:rowb])
                else:
                    qe.dma_start(w_t[:], w[:, k, :])

                if k < kc - 2:
                    for coff, cw in slices:
                        mm(k, w_t, coff, cw, start=(k == 0), stop=False)
                elif k == kc - 2:
                    w_prev = w_t  # final two chunks run per-PSUM-tile below
                else:
                    # final two chunks: finish per PSUM tile, evict to fp16
                    # and store while the remaining tiles' matmuls drain;
                    # casts alternate DVE/Act so two evict chains run in
                    # parallel with the tail of the matmul stream
                    for t, (toff, twd) in enumerate(ptiles):
                        for kk, wt in ((k - 1, w_prev), (k, w_t)):
                            for coff, cw in slices:
                                if toff <= coff < toff + twd:
                                    mm(kk, wt, coff, cw, start=False,
                                       stop=(kk == k))
                        o_t = opool.tile([128, 1024], f16, tag="o")
                        if t % 2 == 0:
                            nc.vector.tensor_copy(o_t[:, 0:twd], psums[t][:])
                        else:
                            nc.scalar.copy(o_t[:, 0:twd], psums[t][:])
                        q2 = nc.sync if t % 2 == 0 else nc.scalar
                        q2.dma_start(out[:, toff : toff + twd], o_t[:, 0:twd])

    nc.compile()
    _dedup_ldweights(nc)
    return nc


def _dedup_ldweights(nc):
    """Remove InstLdweights that reload the PE array with the exact weights it
    already holds (consecutive matmuls sharing one stationary operand).  The
    tile legalizer emits one LDWEIGHTS per matmul and neither it nor walrus
    dedups, so slice groups sharing a lhsT pay redundant ~100ns array loads
    each -- pure serial PE time.  Safe here because each stationary region is
    written once (per wpool slot generation) before its matmuls.  Any
    waits/updates on a removed LDW are transferred to the next PE inst."""
    import concourse.mybir as mybir

    for blk in nc.m.functions[0].blocks:
        insts = blk.instructions
        loaded = None
        pending = []  # sync infos of removed LDWs, to merge into next PE inst
        idx = 0
        while idx < len(insts):
            inst = insts[idx]
            if isinstance(inst, mybir.InstLdweights):
                key = (
                    str(inst.ins[0]),
                    str(inst.tile_position),
                    str(inst.perf_mode),
                    str(inst.is_transpose),
                )
                if loaded == key:
                    si = inst.sync_info
                    if si is not None and (si.on_wait or si.on_update):
                        pending.append(si)
                    del insts[idx]
                    continue
                loaded = key
            elif isinstance(inst, mybir.InstMatmult) and pending:
                si = inst.sync_info
                if si is None:
                    si = mybir.SyncInfo(on_wait=[], on_update=[])
                for p in pending:
                    si.on_wait = list(si.on_wait) + list(p.on_wait)
                    si.on_update = list(si.on_update) + list(p.on_update)
                inst.sync_info = si
                pending = []
            idx += 1
        assert not pending, "dangling sync from removed LDWEIGHTS"


def _get_compiled(kc, ncol):
    if (kc, ncol) not in _compiled:
        _compiled[(kc, ncol)] = _build_bass(kc, ncol)
    return _compiled[(kc, ncol)]


def _prep_cores(features, unroll_mat, occurrences, dst_masks):
    """Host-side prep: mask-gather W rows, drop all-zero rows AND columns,
    pack fp16 features^T chunk in front of each fp8 W chunk row."""
    f8 = ml_dtypes.float8_e4m3
    per = []
    for b in range(B):
        wg = unroll_mat[b][dst_masks[b]]          # [E, U] f32, entries 0/1
        keep = wg.any(axis=1)                      # drop rows with no targets
        wk = wg[keep]
        fk = features[b][:, keep]                  # matching feature columns
        colidx = np.where(wk.any(axis=0))[0]       # drop all-zero output cols
        per.append((wk[:, colidx], fk, colidx))
    rmax = max(w_.shape[0] for w_, _, _ in per)
    cmax = max(w_.shape[1] for w_, _, _ in per)
    kc = (rmax + 127) // 128
    e = kc * 128
    ncol = ((cmax + 31) // 32) * 32
    rowb = AB + ncol

    in_maps, meta = [], []
    for b in range(B):
        wkc, fk, colidx = per[b]
        r, c = wkc.shape
        at = np.zeros((e, 128), dtype=np.float32)  # A^T, zero-padded rows
        at[:r] = fk.T
        # a3[p, k, m] = at[k*128+p, m] as fp16 -> 256 bytes per (p, k)
        a3 = np.ascontiguousarray(
            at.astype(np.float16).reshape(kc, 128, 128).transpose(1, 0, 2)
        )
        wpad = np.zeros((e, ncol), dtype=f8)
        wpad[:r, :c] = wkc.astype(f8)              # 0/1 -> exact even in fp8
        w3 = np.ascontiguousarray(wpad.reshape(kc, 128, ncol).transpose(1, 0, 2))
        packed = np.empty((128, kc, rowb), dtype=np.uint8)
        packed[:, :, :AB] = a3.view(np.uint8).reshape(128, kc, AB)
        packed[:, :, AB:] = w3.view(np.uint8)
        in_maps.append({"w": packed.view(ml_dtypes.float8_e4m3)})
        meta.append((colidx, c))
    return kc, ncol, in_maps, meta


def kernel(features, unroll_mat, occurrences, dst_masks):
    import concourse.bass_utils as bass_utils

    features = np.asarray(features, dtype=np.float32)
    unroll_mat = np.asarray(unroll_mat, dtype=np.float32)
    occurrences = np.asarray(occurrences, dtype=np.float32)
    dst_masks = np.asarray(dst_masks).astype(bool)

    kc, ncol, in_maps, meta = _prep_cores(features, unroll_mat, occurrences, dst_masks)
    nc = _get_compiled(kc, ncol)
    try:
        res = bass_utils.run_bass_kernel_spmd(
            nc, in_maps, core_ids=list(range(NCORES))
        )
    except Exception:
        # one retry for transient device hiccups (e.g. a wedged exec unit)
        res = bass_utils.run_bass_kernel_spmd(
            nc, in_maps, core_ids=list(range(NCORES))
        )
    occ = occurrences.reshape(B, U)
    full = np.zeros((B, NF, U), dtype=np.float32)
    for b in range(B):
        colidx, c = meta[b]
        dev = np.asarray(res.results[b]["out"])[:, :c].astype(np.float32)
        full[b][:, colidx] = dev / occ[b, colidx][None, :]
    return full


# revision 15
# speedup vs baseline: 1.0793x; 1.0793x over previous
# Trainium2 Bass kernel for nn_MeshUnpool (gnn_message_passing).
#
# Reference semantics (per mesh b):
#   idx = cumsum(dst_mask)-1 at true slots; padded[v,:] = mask[v] ? features[:,idx[v]] : 0
#   out = (unroll_mat[b].T @ padded).T / occ  ==  (features[b] @ unroll_mat[b][mask_rows]) / occ
# i.e. the gather+scatter collapses to selecting the E=3072 masked rows of
# unroll_mat, leaving a dense [NF,E] @ [E,U] matmul per mesh, divided
# column-wise by occurrences.  Pure data parallel: one mesh per core.
#
# On-device compute per core:
#   out[128, ncol] = sum_k A[k].T @ W[k]
# A = features^T in fp16 (stationary), W = the masked-row gather of
# unroll_mat cast to fp8 (entries 0/1, lossless; moving operand at 1
# byte/elem).  The PE runs 1 cycle per output column per 128-row chunk for
# any <=16-bit dtype pair, so a single fp16 pass costs half of what the old
# bf16 hi/lo split paid.
#
# Memory-bound regime, ~12.4 MB/core HBM traffic:
#   - all-zero ROWS of W dropped on host (~6%)
#   - all-zero COLUMNS of W dropped on host (~11%); host scatters computed
#     columns back into the full [128, 4096] output
#   - occurrences never shipped: division happens on host after the gather
#   - out ships fp16 (0.92 MB)
#   - each chunk's 256-byte-per-partition fp16 stationary is PACKED IN FRONT
#     of its W rows and read through a bitcast AP, so one DMA per chunk
#     delivers both and the tile scheduler cannot reorder A behind W
#   - the W stream alternates between the two HWDGE queues (SP + Act)
#   - ~2.5us of warmup matmuls on a zeroed tile ramp the PE out of its cold
#     p-state while the first W chunk streams in
#   - redundant LDWEIGHTS are stripped post-compile (_dedup_ldweights)

import numpy as np
import ml_dtypes

B, NF, E, U = 8, 128, 3072, 4096
NCORES = 8
AB = 256  # stationary bytes per partition packed ahead of each W chunk row

_compiled = {}


def _build_bass(kc, ncol):
    """Per-core program: kc 128-row chunks contracted over ncol kept output
    columns, one fp16(stationary) x fp8(moving) matmul pass per chunk."""
    import concourse.bass as bass
    import concourse.bacc as bacc
    import concourse.mybir as mybir
    import concourse.tile as tile

    nc = bacc.Bacc("TRN2", target_bir_lowering=False, debug=False)
    fp8 = mybir.dt.float8e4
    f16 = mybir.dt.float16
    f32 = mybir.dt.float32

    rowb = AB + ncol  # bytes per partition per chunk: [fp16 A | fp8 W]
    w = nc.dram_tensor("w", [128, kc, rowb], fp8, kind="ExternalInput").ap()
    out = nc.dram_tensor("out", [128, ncol], f16, kind="ExternalOutput").ap()

    # PSUM column tiles of up to 1024 (2 banks); matmul slices of up to 512
    # (1 bank) never cross a bank boundary.
    ptiles = []
    off = 0
    while off < ncol:
        wd = min(1024, ncol - off)
        ptiles.append((off, wd))
        off += wd
    slices = []
    off = 0
    while off < ncol:
        wd = min(512, ncol - off)
        slices.append((off, wd))
        off += wd

    def locate(coff):
        for i, (o, wd) in enumerate(ptiles):
            if o <= coff < o + wd:
                return i, coff - o
        raise AssertionError(coff)

    with tile.TileContext(nc) as tc:
        with (
            tc.tile_pool(name="zpool", bufs=1) as zpool,
            tc.tile_pool(name="wpool", bufs=6) as wpool,
            tc.tile_pool(name="psum", bufs=1, space=bass.MemorySpace.PSUM) as ppool,
            tc.tile_pool(name="opool", bufs=4) as opool,
        ):
            z_t = zpool.tile([128, 512], fp8, tag="z")
            psums = [
                ppool.tile([128, wd], f32, tag=f"ps{i}", name=f"ps{i}")
                for i, (o, wd) in enumerate(ptiles)
            ]

            # PE p-state warmup: ~5us of throwaway matmuls on a zeroed tile
            # run while the W prologue streams (the first chunk takes that
            # long to arrive anyway: trigger chain + DGE ramp + sem prop), so
            # the real stream starts at the full 2.4 GHz clock and never
            # exposes the cold 0.65 GHz p-state.
            nc.vector.memset(z_t[:], 0)
            for _ in range(8):
                nc.tensor.matmul(
                    psums[0][:, 0:512], z_t[:, 0:128], z_t[:], start=True, stop=True
                )

            def mm(k, w_t, coff, cw, start, stop):
                ti, lo = locate(coff)
                nc.tensor.matmul(
                    psums[ti][:, lo : lo + cw],
                    w_t[:, 0:AB].bitcast(f16),
                    w_t[:, AB + coff : AB + coff + cw],
                    start=start,
                    stop=stop,
                )

            for k in range(kc):
                w_t = wpool.tile([128, rowb], fp8, tag="w")
                if k == 0:
                    # prologue: first chunk split so the first matmul waits on
                    # only ~96KB (stationary + first 512 W columns)
                    c1, c2 = AB + 512, AB + 1536
                    nc.sync.dma_start(w_t[:, 0:c1], w[:, 0, 0:c1])
                    nc.sync.dma_start(w_t[:, c1:c2], w[:, 0, c1:c2])
                    nc.sync.dma_start(w_t[:, c2:rowb], w[:, 0, c2:rowb])
                elif k % 3 == 1:
                    nc.scalar.dma_start(w_t[:], w[:, k, :])
                elif k % 3 == 2:
                    nc.gpsimd.dma_start(w_t[:], w[:, k, :])
                else:
                    nc.sync.dma_start(w_t[:], w[:, k, :])

                if k < kc - 2:
                    for coff, cw in slices:
                        mm(k, w_t, coff, cw, start=(k == 0), stop=False)
                elif k == kc - 2:
                    w_prev = w_t  # final two chunks run per-PSUM-tile below
                else:
                    # final two chunks: finish per PSUM tile, evict to fp16
                    # and store while the remaining tiles' matmuls drain;
                    # casts alternate DVE/Act so two evict chains run in
                    # parallel with the tail of the matmul stream
                    for t, (toff, twd) in enumerate(ptiles):
                        for kk, wt in ((k - 1, w_prev), (k, w_t)):
                            for coff, cw in slices:
                                if toff <= coff < toff + twd:
                                    mm(kk, wt, coff, cw, start=False,
                                       stop=(kk == k))
                        o_t = opool.tile([128, 1024], f16, tag="o")
                        if t % 2 == 0:
                            nc.vector.tensor_copy(o_t[:, 0:twd], psums[t][:])
                        else:
                            nc.scalar.copy(o_t[:, 0:twd], psums[t][:])
                        q2 = nc.sync if t % 2 == 0 else nc.scalar
                        q2.dma_start(out[:, toff : toff + twd], o_t[:, 0:twd])

    nc.compile()
    _dedup_ldweights(nc)
    return nc


def _dedup_ldweights(nc):
    """Remove InstLdweights that reload the PE array with the exact weights it
    already holds (consecutive matmuls sharing one stationary operand).  The
    tile legalizer emits one LDWEIGHTS per matmul and neither it nor walrus
    dedups, so slice groups sharing a lhsT pay redundant ~100ns array loads
    each -- pure serial PE time.  Safe here because each stationary region is
    written once (per wpool slot generation) before its matmuls.  Any
    waits/updates on a removed LDW are transferred to the next PE inst."""
    import concourse.mybir as mybir

    for blk in nc.m.functions[0].blocks:
        insts = blk.instructions
        loaded = None
        pending = []  # sync infos of removed LDWs, to merge into next PE inst
        idx = 0
        while idx < len(insts):
            inst = insts[idx]
            if isinstance(inst, mybir.InstLdweights):
                key = (
                    str(inst.ins[0]),
                    str(inst.tile_position),
                    str(inst.perf_mode),
                    str(inst.is_transpose),
                )
                if loaded == key:
                    si = inst.sync_info
                    if si is not None and (si.on_wait or si.on_update):
                        pending.append(si)
                    del insts[idx]
                    continue
                loaded = key
            elif isinstance(inst, mybir.InstMatmult) and pending:
                si = inst.sync_info
                if si is None:
                    si = mybir.SyncInfo(on_wait=[], on_update=[])
                for p in pending:
                    si.on_wait = list(si.on_wait) + list(p.on_wait)
                    si.on_update = list(si.on_update) + list(p.on_update)
                inst.sync_info = si
                pending = []
            idx += 1
        assert not pending, "dangling sync from removed LDWEIGHTS"


def _get_compiled(kc, ncol):
    if (kc, ncol) not in _compiled:
        _compiled[(kc, ncol)] = _build_bass(kc, ncol)
    return _compiled[(kc, ncol)]


def _prep_cores(features, unroll_mat, occurrences, dst_masks):
    """Host-side prep: mask-gather W rows, drop all-zero rows AND columns,
    pack fp16 features^T chunk in front of each fp8 W chunk row."""
    f8 = ml_dtypes.float8_e4m3
    per = []
    for b in range(B):
        wg = unroll_mat[b][dst_masks[b]]          # [E, U] f32, entries 0/1
        keep = wg.any(axis=1)                      # drop rows with no targets
        wk = wg[keep]
        fk = features[b][:, keep]                  # matching feature columns
        colidx = np.where(wk.any(axis=0))[0]       # drop all-zero output cols
        per.append((wk[:, colidx], fk, colidx))
    rmax = max(w_.shape[0] for w_, _, _ in per)
    cmax = max(w_.shape[1] for w_, _, _ in per)
    kc = (rmax + 127) // 128
    e = kc * 128
    ncol = ((cmax + 31) // 32) * 32
    rowb = AB + ncol

    in_maps, meta = [], []
    for b in range(B):
        wkc, fk, colidx = per[b]
        r, c = wkc.shape
        at = np.zeros((e, 128), dtype=np.float32)  # A^T, zero-padded rows
        at[:r] = fk.T
        # a3[p, k, m] = at[k*128+p, m] as fp16 -> 256 bytes per (p, k)
        a3 = np.ascontiguousarray(
            at.astype(np.float16).reshape(kc, 128, 128).transpose(1, 0, 2)
        )
        wpad = np.zeros((e, ncol), dtype=f8)
        wpad[:r, :c] = wkc.astype(f8)              # 0/1 -> exact even in fp8
        w3 = np.ascontiguousarray(wpad.reshape(kc, 128, ncol).transpose(1, 0, 2))
        packed = np.empty((128, kc, rowb), dtype=np.uint8)
        packed[:, :, :AB] = a3.view(np.uint8).reshape(128, kc, AB)
        packed[:, :, AB:] = w3.view(np.uint8)
        in_maps.append({"w": packed.view(ml_dtypes.float8_e4m3)})
        meta.append((colidx, c))
    return kc, ncol, in_maps, meta


def kernel(features, unroll_mat, occurrences, dst_masks):
    import concourse.bass_utils as bass_utils

    features = np.asarray(features, dtype=np.float32)
    unroll_mat = np.asarray(unroll_mat, dtype=np.float32)
    occurrences = np.asarray(occurrences, dtype=np.float32)
    dst_masks = np.asarray(dst_masks).astype(bool)

    kc, ncol, in_maps, meta = _prep_cores(features, unroll_mat, occurrences, dst_masks)
    nc = _get_compiled(kc, ncol)
    try:
        res = bass_utils.run_bass_kernel_spmd(
            nc, in_maps, core_ids=list(range(NCORES))
        )
    except Exception:
        # one retry for transient device hiccups (e.g. a wedged exec unit)
        res = bass_utils.run_bass_kernel_spmd(
            nc, in_maps, core_ids=list(range(NCORES))
        )
    occ = occurrences.reshape(B, U)
    full = np.zeros((B, NF, U), dtype=np.float32)
    for b in range(B):
        colidx, c = meta[b]
        dev = np.asarray(res.results[b]["out"])[:, :c].astype(np.float32)
        full[b][:, colidx] = dev / occ[b, colidx][None, :]
    return full
